# revision 1
# baseline (speedup 1.0000x reference)
import sys

sys.path.insert(0, "/opt/trn_rl_repo")

import numpy as np

# model dims (hardcoded per problem spec)
B = 2
L_IN = 135000
D_MODEL = 768
N_LAYERS = 24
D_INNER = 1536
D_STATE = 16
D_CONV = 4
DT_RANK = 48
K_DS = 32
S_DS = 32
LATENT = 64
T = (L_IN - K_DS) // S_DS + 1  # 4218

N_CORES = 8
G = 4                 # d_inner shards per batch group
TC = 512              # time chunk
SG = 2                # s-planes per scan group
P = 128


def _chunks(total, step):
    out = []
    t0 = 0
    while t0 < total:
        out.append((t0, min(step, total - t0)))
        t0 += step
    return out


def build_bass(n_layers, t_len, d_sh, replica_groups, use_cc, a_const=None):
    """Build the SPMD bass program (identical on every core; data differs)."""
    import concourse.bass as bass
    import concourse.bacc as bacc
    import concourse.tile as tile
    from concourse import mybir

    F32 = mybir.dt.float32
    BF16 = mybir.dt.bfloat16
    AF = mybir.ActivationFunctionType
    OP = mybir.AluOpType

    DB = d_sh // P            # d-blocks per core
    NSG = D_STATE // SG       # scan groups
    SEG = SG * DB             # scan segments per group tile
    chunks = _chunks(t_len, TC)
    NCH = len(chunks)
    MB = D_MODEL // P         # 6

    nc = bacc.Bacc("TRN2", target_bir_lowering=False, debug=True)

    # ---------------- DRAM inputs ----------------
    xds_d = nc.dram_tensor("xds", [K_DS, t_len], F32, kind="ExternalInput")
    dsw_d = nc.dram_tensor("dsw", [K_DS, D_MODEL], F32, kind="ExternalInput")
    dsb_d = nc.dram_tensor("dsb", [D_MODEL, 1], F32, kind="ExternalInput")
    inw_d = nc.dram_tensor("inw", [n_layers, D_MODEL, 2 * d_sh], BF16, kind="ExternalInput")
    xpw_d = nc.dram_tensor("xpw", [n_layers, d_sh, 96], BF16, kind="ExternalInput")
    dtw_d = nc.dram_tensor("dtw", [n_layers, DT_RANK, d_sh], BF16, kind="ExternalInput")
    outw_d = nc.dram_tensor("outw", [n_layers, d_sh, D_MODEL], BF16, kind="ExternalInput")
    cvw_d = nc.dram_tensor("cvw", [n_layers, d_sh, D_CONV], F32, kind="ExternalInput")
    cvb_d = nc.dram_tensor("cvb", [n_layers, d_sh, 1], F32, kind="ExternalInput")
    dtb_d = nc.dram_tensor("dtb", [n_layers, d_sh, 1], F32, kind="ExternalInput")
    acol_d = nc.dram_tensor("acol", [n_layers, d_sh, D_STATE], F32, kind="ExternalInput")
    dsk_d = nc.dram_tensor("dsk", [n_layers, d_sh, 1], BF16, kind="ExternalInput")
    qid_d = nc.dram_tensor("qid", [P, P], F32, kind="ExternalInput")   # I / n_shards
    idb_d = nc.dram_tensor("idb", [P, P], BF16, kind="ExternalInput")  # identity
    ones_d = nc.dram_tensor("ones", [1, P], BF16, kind="ExternalInput")
    onec_d = nc.dram_tensor("onec", [P, 1], BF16, kind="ExternalInput")
    pjw_d = nc.dram_tensor("pjw", [D_MODEL, LATENT], F32, kind="ExternalInput")
    pjb_d = nc.dram_tensor("pjb", [LATENT, 1], F32, kind="ExternalInput")
    lnw_d = nc.dram_tensor("lnw", [1, LATENT], F32, kind="ExternalInput")
    lnb_d = nc.dram_tensor("lnb", [1, LATENT], F32, kind="ExternalInput")

    out_d = nc.dram_tensor("head_out", [1, LATENT], F32, kind="ExternalOutput")

    with tile.TileContext(nc) as tc:
        import contextlib
        with contextlib.ExitStack() as ctx:
            wpool = ctx.enter_context(tc.tile_pool(name="wpool", bufs=2))
            cpool = ctx.enter_context(tc.tile_pool(name="cpool", bufs=1))
            work = ctx.enter_context(tc.tile_pool(name="work", bufs=2))
            lat = ctx.enter_context(tc.tile_pool(name="lat", bufs=2))
            spool = ctx.enter_context(tc.tile_pool(name="spool", bufs=1))
            mm = ctx.enter_context(tc.tile_pool(name="mm", bufs=2, space="PSUM"))
            sm = ctx.enter_context(tc.tile_pool(name="sm", bufs=1, space="PSUM"))
            ypsum = ctx.enter_context(tc.tile_pool(name="ypsum", bufs=1, space="PSUM"))
            dram = ctx.enter_context(tc.tile_pool(name="dram", bufs=1, space="DRAM"))

            # persistent constants
            idb = cpool.tile([P, P], BF16)
            nc.sync.dma_start(out=idb, in_=idb_d[:, :])
            qid = cpool.tile([P, P], F32)
            nc.sync.dma_start(out=qid, in_=qid_d[:, :])
            ones_r = cpool.tile([1, P], BF16)
            nc.sync.dma_start(out=ones_r, in_=ones_d[:, :])
            onec = cpool.tile([P, 1], BF16)
            nc.sync.dma_start(out=onec, in_=onec_d[:, :])
            epst = cpool.tile([P, 1], F32)
            nc.vector.memset(epst, 1e-5)

            cc_in = [dram.tile([NCH, D_MODEL, TC], F32, name=f"cc_in{i}")
                     for i in range(2)]
            cc_out = [dram.tile([NCH, D_MODEL, TC], F32, name=f"cc_out{i}")
                      for i in range(2)]
            bc_dram = dram.tile([2 * D_STATE, t_len], BF16)
            tail_t0, tail_tw = chunks[-1]
            if tail_tw < TC:
                zt = work.tile([P, TC], F32, tag="dsout")
                nc.vector.memset(zt, 0.0)
                for bix in range(2):
                    for mb in range(MB):
                        nc.sync.dma_start(
                            out=cc_in[bix][NCH - 1, mb * P:(mb + 1) * P, tail_tw:TC],
                            in_=zt[:, :TC - tail_tw])
            zb_dram = dram.tile([LATENT, 1], F32)

            def rmsnorm_rs(h_c, tw, tag_suffix=""):
                """h_c [P, MB, tw] f32 -> rs broadcast psum tile [P, tw]."""
                sq = work.tile([P, MB, TC], BF16, tag="nrm")
                nc.scalar.activation(sq[:, :, :tw], h_c[:, :, :tw], AF.Square)
                msq = sm.tile([1, TC], F32, tag="msq")
                for kb in range(MB):
                    nc.tensor.matmul(msq[:, :tw], onec, sq[:, kb, :tw],
                                     start=(kb == 0), stop=(kb == MB - 1))
                lnm = work.tile([1, TC], F32, tag="lnm")
                nc.scalar.activation(lnm[:, :tw], msq[:, :tw], AF.Ln,
                                     bias=epst[:1, :], scale=1.0 / D_MODEL)
                rsr = work.tile([1, TC], BF16, tag="rsr")
                nc.scalar.activation(rsr[:, :tw], lnm[:, :tw], AF.Exp,
                                     bias=0.0, scale=-0.5)
                rsb = sm.tile([P, TC], F32, tag="rsb")
                nc.tensor.matmul(rsb[:, :tw], ones_r, rsr[:, :tw], start=True, stop=True)
                return rsb

            # ---------------- downsample conv -> cc_out[1] ----------------
            dsw = cpool.tile([K_DS, D_MODEL], F32)
            nc.sync.dma_start(out=dsw, in_=dsw_d[:, :])
            dsb = cpool.tile([P, MB], F32)
            nc.sync.dma_start(out=dsb, in_=dsb_d.rearrange("(mb p) one -> p (mb one)", p=P))
            for ci, (t0, tw) in enumerate(chunks):
                xc = work.tile([K_DS, TC], F32, tag="dsout")
                nc.sync.dma_start(out=xc[:, :tw], in_=xds_d[:, t0:t0 + tw])
                for mb in range(MB):
                    ps = mm.tile([P, TC], F32, tag="mm")
                    nc.tensor.matmul(ps[:, :tw], dsw[:, mb * P:(mb + 1) * P],
                                     xc[:, :tw], start=True, stop=True)
                    hsb = work.tile([P, TC], F32, tag="dsout")
                    nc.scalar.activation(hsb[:, :tw], ps[:, :tw], AF.Identity,
                                         bias=dsb[:, mb:mb + 1], scale=1.0)
                    nc.sync.dma_start(out=cc_out[1][ci, mb * P:(mb + 1) * P, :tw],
                                      in_=hsb[:, :tw])

            # ---------------- layers ----------------
            for l in range(n_layers):
                pp = l % 2
                prev = 1 - pp

                inw = wpool.tile([P, MB, 2 * d_sh], BF16, tag="inw")
                nc.sync.dma_start(out=inw, in_=inw_d[l].rearrange("(kb p) m -> p kb m", p=P))
                xpw = wpool.tile([P, DB, 96], BF16, tag="xpw")
                nc.sync.dma_start(out=xpw, in_=xpw_d[l].rearrange("(kb p) m -> p kb m", p=P))
                dtw = wpool.tile([DT_RANK, d_sh], BF16, tag="dtw")
                nc.sync.dma_start(out=dtw, in_=dtw_d[l, :, :])
                outw = wpool.tile([P, DB, D_MODEL], BF16, tag="outw")
                nc.sync.dma_start(out=outw, in_=outw_d[l].rearrange("(kb p) m -> p kb m", p=P))
                cvw = wpool.tile([P, DB, D_CONV], F32, tag="cvw")
                nc.sync.dma_start(out=cvw, in_=cvw_d[l].rearrange("(kb p) m -> p kb m", p=P))
                cvb = wpool.tile([P, DB], F32, tag="cvb")
                nc.sync.dma_start(out=cvb, in_=cvb_d[l].rearrange("(kb p) one -> p (kb one)", p=P))
                dtb = wpool.tile([P, DB], F32, tag="dtb")
                nc.sync.dma_start(out=dtb, in_=dtb_d[l].rearrange("(kb p) one -> p (kb one)", p=P))
                dsk = wpool.tile([P, DB], BF16, tag="dsk")
                nc.sync.dma_start(out=dsk, in_=dsk_d[l].rearrange("(kb p) one -> p (kb one)", p=P))
                if a_const is None:
                    acol = wpool.tile([P, DB, D_STATE], F32, tag="acol")
                    nc.sync.dma_start(out=acol,
                                      in_=acol_d[l].rearrange("(kb p) s -> p kb s", p=P))

                state = spool.tile([P, D_STATE * DB], F32, tag="state")
                nc.vector.memset(state, 0.0)

                prev_uraw = None
                for ci, (t0, tw) in enumerate(chunks):
                    # ---- A1: load h chunk (residual already folded in) ----
                    h_c = work.tile([P, MB, TC], F32, tag="h_c")
                    nc.sync.dma_start(out=h_c[:, :, :tw], in_=cc_out[prev][ci, :, :tw]
                                      .rearrange("(mb p) t -> p mb t", p=P))
                    # ---- A2: rmsnorm ----
                    rsb = rmsnorm_rs(h_c, tw)
                    normed = work.tile([P, MB, TC], BF16, tag="nrm")
                    rsb3 = bass.AP(tensor=rsb.tensor, offset=rsb.offset,
                                   ap=[rsb.ap[0], [0, MB], [1, tw]])
                    nc.vector.tensor_tensor(normed[:, :, :tw], h_c[:, :, :tw], rsb3,
                                            op=OP.mult)
                    # ---- A3: in_proj ----
                    uraw = work.tile([P, DB, D_CONV - 1 + TC], BF16, tag="uraw")
                    if ci == 0:
                        nc.vector.memset(uraw[:, :, 0:D_CONV - 1], 0.0)
                    else:
                        pw = chunks[ci - 1][1]
                        nc.vector.tensor_copy(uraw[:, :, 0:D_CONV - 1],
                                              prev_uraw[:, :, pw:pw + D_CONV - 1])
                    siluz = work.tile([P, DB, TC], BF16, tag="siluz")
                    for mb in range(2 * DB):
                        psm = mm.tile([P, TC], F32, tag="mm")
                        for kb in range(MB):
                            nc.tensor.matmul(psm[:, :tw],
                                             inw[:, kb, mb * P:(mb + 1) * P],
                                             normed[:, kb, :tw],
                                             start=(kb == 0), stop=(kb == MB - 1))
                        if mb < DB:
                            nc.scalar.copy(uraw[:, mb, D_CONV - 1:D_CONV - 1 + tw],
                                           psm[:, :tw])
                        else:
                            db = mb - DB
                            zsg = work.tile([P, TC], BF16, tag="zsg")
                            nc.scalar.activation(zsg[:, :tw], psm[:, :tw], AF.Sigmoid)
                            zbf = work.tile([P, TC], BF16, tag="zbf")
                            nc.scalar.copy(zbf[:, :tw], psm[:, :tw])
                            nc.vector.tensor_tensor(siluz[:, db, :tw], zbf[:, :tw],
                                                    zsg[:, :tw], op=OP.mult)
                    # ---- A4: causal depthwise conv + silu ----
                    u_f = work.tile([P, DB, TC], BF16, tag="u_f")
                    for db in range(DB):
                        ucv = work.tile([P, TC], BF16, tag="ucv")
                        nc.vector.tensor_scalar(ucv[:, :tw], uraw[:, db, 0:tw],
                                                cvw[:, db, 0:1], None, op0=OP.mult)
                        for j in range(1, D_CONV):
                            nc.vector.scalar_tensor_tensor(
                                ucv[:, :tw], uraw[:, db, j:j + tw], cvw[:, db, j:j + 1],
                                ucv[:, :tw], op0=OP.mult, op1=OP.add)
                        ucb = work.tile([P, TC], BF16, tag="ucb")
                        nc.scalar.activation(ucb[:, :tw], ucv[:, :tw], AF.Identity,
                                             bias=cvb[:, db:db + 1], scale=1.0)
                        usg = work.tile([P, TC], BF16, tag="usg")
                        nc.scalar.activation(usg[:, :tw], ucv[:, :tw], AF.Sigmoid,
                                             bias=cvb[:, db:db + 1], scale=1.0)
                        nc.vector.tensor_tensor(u_f[:, db, :tw], ucb[:, :tw],
                                                usg[:, :tw], op=OP.mult)
                    # ---- A5: x_proj ----
                    psx = mm.tile([P, TC], F32, tag="mm")
                    for kb in range(DB):
                        nc.tensor.matmul(psx[0:96, :tw], xpw[:, kb, :], u_f[:, kb, :tw],
                                         start=(kb == 0), stop=(kb == DB - 1))
                    dtv = work.tile([64, TC], BF16, tag="dtv")
                    nc.scalar.copy(dtv[:, :tw], psx[0:64, :tw])
                    bcr = work.tile([2 * D_STATE, TC], BF16, tag="bcr")
                    nc.scalar.copy(bcr[:, :tw], psx[64:96, :tw])
                    nc.sync.dma_start(out=bc_dram[:, t0:t0 + tw], in_=bcr[:, :tw])
                    # ---- A6: dt_proj -> delta (softplus), w = delta*u ----
                    dlt = work.tile([P, DB, TC], BF16, tag="dlt")
                    for db in range(DB):
                        psd = mm.tile([P, TC], F32, tag="mm")
                        nc.tensor.matmul(psd[:, :tw], dtw[:, db * P:(db + 1) * P],
                                         dtv[0:DT_RANK, :tw], start=True, stop=True)
                        esb = work.tile([P, TC], F32, tag="esb")
                        nc.scalar.activation(esb[:, :tw], psd[:, :tw], AF.Exp,
                                             bias=dtb[:, db:db + 1], scale=1.0)
                        nc.scalar.activation(dlt[:, db, :tw], esb[:, :tw], AF.Ln,
                                             bias=1.0, scale=1.0)
                    wmu = work.tile([P, DB, TC], BF16, tag="wmu")
                    nc.vector.tensor_tensor(wmu[:, :, :tw], dlt[:, :, :tw],
                                            u_f[:, :, :tw], op=OP.mult)
                    # ---- A7: scan ----
                    yp = ypsum.tile([P, DB * TC], F32, tag="yp")
                    for g in range(NSG):
                        s0 = g * SG
                        bbc = lat.tile([P, SG, TC], BF16, tag="bbc")
                        nc.sync.dma_start(
                            out=bbc[:, :, :tw],
                            in_=bass.AP(tensor=bc_dram.tensor,
                                        offset=bc_dram.offset + s0 * t_len + t0,
                                        ap=[[0, P], [t_len, SG], [1, tw]]))
                        cbc = lat.tile([P, SG, TC], BF16, tag="cbc")
                        nc.sync.dma_start(
                            out=cbc[:, :, :tw],
                            in_=bass.AP(tensor=bc_dram.tensor,
                                        offset=bc_dram.offset + (D_STATE + s0) * t_len + t0,
                                        ap=[[0, P], [t_len, SG], [1, tw]]))
                        av = lat.tile([P, SG, DB, 1 + TC], F32, tag="av")
                        nc.vector.memset(av[:, :, :, 0:1], 0.0)
                        for si in range(SG):
                            s = s0 + si
                            if a_const is not None:
                                nc.scalar.activation(av[:, si, :, 1:1 + tw],
                                                     dlt[:, :, :tw], AF.Exp,
                                                     bias=0.0, scale=float(a_const[l][s]))
                            else:
                                for db in range(DB):
                                    nc.scalar.activation(av[:, si, db, 1:1 + tw],
                                                         dlt[:, db, :tw], AF.Exp,
                                                         bias=0.0,
                                                         scale=acol[:, db, s:s + 1])
                        bv = lat.tile([P, SG, DB, 1 + TC], BF16, tag="bv")
                        st_ap = bass.AP(tensor=state.tensor, offset=state.offset + s0 * DB,
                                        ap=[state.ap[0], [DB, SG], [1, DB], [0, 1]])
                        nc.vector.tensor_copy(bv[:, :, :, 0:1], st_ap)
                        wb = bass.AP(tensor=wmu.tensor, offset=wmu.offset,
                                     ap=[wmu.ap[0], [0, SG], [TC, DB], [1, tw]])
                        bb = bass.AP(tensor=bbc.tensor, offset=bbc.offset,
                                     ap=[bbc.ap[0], [TC, SG], [0, DB], [1, tw]])
                        nc.vector.tensor_tensor(bv[:, :, :, 1:1 + tw], wb, bb, op=OP.mult)
                        hv = lat.tile([P, SG, DB, 1 + TC], BF16, tag="hv")
                        if tw == TC:
                            flat = SEG * (1 + TC)
                            sc_a = bass.AP(tensor=av.tensor, offset=av.offset,
                                           ap=[av.ap[0], [1, flat]])
                            sc_b = bass.AP(tensor=bv.tensor, offset=bv.offset,
                                           ap=[bv.ap[0], [1, flat]])
                            sc_h = bass.AP(tensor=hv.tensor, offset=hv.offset,
                                           ap=[hv.ap[0], [1, flat]])
                            nc.vector.tensor_tensor_scan(sc_h, sc_a, sc_b, 0.0,
                                                         op0=OP.mult, op1=OP.add)
                        else:
                            for si in range(SG):
                                for db in range(DB):
                                    nc.vector.tensor_tensor_scan(
                                        hv[:, si, db, 0:1 + tw],
                                        av[:, si, db, 0:1 + tw],
                                        bv[:, si, db, 0:1 + tw], 0.0,
                                        op0=OP.mult, op1=OP.add)
                        st_ap2 = bass.AP(tensor=state.tensor, offset=state.offset + s0 * DB,
                                         ap=[state.ap[0], [DB, SG], [1, DB], [0, 1]])
                        nc.vector.tensor_copy(st_ap2, hv[:, :, :, tw:tw + 1])
                        yc = lat.tile([P, SG, DB, TC], BF16, tag="bv")
                        cb3 = bass.AP(tensor=cbc.tensor, offset=cbc.offset,
                                      ap=[cbc.ap[0], [TC, SG], [0, DB], [1, tw]])
                        nc.vector.tensor_tensor(yc[:, :, :, :tw], hv[:, :, :, 1:1 + tw],
                                                cb3, op=OP.mult)
                        for si in range(SG):
                            s = s0 + si
                            for db in range(DB):
                                nc.tensor.matmul(
                                    yp[:, db * TC:db * TC + tw], idb,
                                    yc[:, si, db, :tw],
                                    start=(s == 0), stop=(s == D_STATE - 1),
                                    skip_group_check=True)
                    # ---- A8: gating ----
                    yg = work.tile([P, DB, TC], BF16, tag="dlt")
                    for db in range(DB):
                        g1 = work.tile([P, TC], F32, tag="esb")
                        nc.vector.scalar_tensor_tensor(
                            g1[:, :tw], u_f[:, db, :tw], dsk[:, db:db + 1],
                            yp[:, db * TC:db * TC + tw], op0=OP.mult, op1=OP.add)
                        nc.vector.tensor_tensor(yg[:, db, :tw], g1[:, :tw],
                                                siluz[:, db, :tw], op=OP.mult)
                    # ---- A9: out_proj + h/G via identity ----
                    for mb in range(MB):
                        pso = mm.tile([P, TC], F32, tag="mm")
                        for kb in range(DB):
                            nc.tensor.matmul(pso[:, :tw],
                                             outw[:, kb, mb * P:(mb + 1) * P],
                                             yg[:, kb, :tw],
                                             start=(kb == 0), stop=False,
                                             skip_group_check=True)
                        nc.tensor.matmul(pso[:, :tw], qid, h_c[:, mb, :tw],
                                         start=False, stop=True, skip_group_check=True)
                        oso = work.tile([P, TC], F32, tag="dsout")
                        nc.scalar.copy(oso[:, :tw], pso[:, :tw])
                        nc.sync.dma_start(out=cc_in[pp][ci, mb * P:(mb + 1) * P, :tw],
                                          in_=oso[:, :tw])
                    if use_cc:
                        nc.gpsimd.collective_compute(
                            "AllReduce", OP.add,
                            replica_groups=replica_groups,
                            ins=[cc_in[pp][ci].opt()],
                            outs=[cc_out[pp][ci].opt()],
                        )
                    else:
                        nc.sync.dma_start(out=cc_out[pp][ci, :, :],
                                          in_=cc_in[pp][ci, :, :])
                    prev_uraw = uraw

            # ---------------- head ----------------
            fin = (n_layers - 1) % 2
            pool_acc = spool.tile([P, MB], F32, tag="pool_acc")
            nc.vector.memset(pool_acc, 0.0)
            for ci, (t0, tw) in enumerate(chunks):
                h_c = work.tile([P, MB, TC], F32, tag="h_c")
                nc.sync.dma_start(out=h_c[:, :, :tw], in_=cc_out[fin][ci, :, :tw]
                                  .rearrange("(mb p) t -> p mb t", p=P))
                rsb = rmsnorm_rs(h_c, tw)
                tmp = work.tile([P, MB, TC], BF16, tag="nrm")
                rsb3 = bass.AP(tensor=rsb.tensor, offset=rsb.offset,
                               ap=[rsb.ap[0], [0, MB], [1, tw]])
                nc.vector.tensor_tensor(tmp[:, :, :tw], h_c[:, :, :tw], rsb3, op=OP.mult)
                pr = work.tile([P, MB], F32, tag="pr")
                nc.vector.tensor_reduce(pr, tmp[:, :, :tw], axis=mybir.AxisListType.X,
                                        op=OP.add)
                nc.vector.tensor_tensor(pool_acc, pool_acc, pr, op=OP.add)
            pjw = cpool.tile([P, MB, LATENT], F32)
            nc.sync.dma_start(out=pjw, in_=pjw_d.rearrange("(mb p) m -> p mb m", p=P))
            pjb = cpool.tile([LATENT, 1], F32)
            nc.sync.dma_start(out=pjb, in_=pjb_d[:, :])
            psz = sm.tile([LATENT, 1], F32, tag="msq")
            for kb in range(MB):
                nc.tensor.matmul(psz, pjw[:, kb, :], pool_acc[:, kb:kb + 1],
                                 start=(kb == 0), stop=(kb == MB - 1))
            zcol = work.tile([LATENT, 1], F32, tag="zcol")
            nc.scalar.activation(zcol, psz, AF.Identity, bias=pjb[:, :], scale=1.0)
            nc.sync.dma_start(out=zb_dram[:, :], in_=zcol)
            zrow = work.tile([1, LATENT], F32, tag="zrow")
            nc.sync.dma_start(out=zrow,
                              in_=bass.AP(tensor=zb_dram.tensor, offset=zb_dram.offset,
                                          ap=[[0, 1], [1, LATENT]]))
            zmu = work.tile([1, 1], F32, tag="zmu")
            nc.vector.tensor_reduce(zmu, zrow, axis=mybir.AxisListType.X, op=OP.add)
            zmm = work.tile([1, 1], F32, tag="zmm")
            nc.scalar.activation(zmm, zmu, AF.Identity, bias=0.0, scale=1.0 / LATENT)
            zc = work.tile([1, LATENT], F32, tag="zc")
            nc.vector.tensor_scalar(zc, zrow, zmm, None, op0=OP.subtract)
            zsq = work.tile([1, LATENT], F32, tag="zsq")
            nc.scalar.activation(zsq, zc, AF.Square)
            zvar = work.tile([1, 1], F32, tag="zvar")
            nc.vector.tensor_reduce(zvar, zsq, axis=mybir.AxisListType.X, op=OP.add)
            zln = work.tile([1, 1], F32, tag="zln")
            nc.scalar.activation(zln, zvar, AF.Ln, bias=epst[:1, :], scale=1.0 / LATENT)
            zrs = work.tile([1, 1], F32, tag="zrs")
            nc.scalar.activation(zrs, zln, AF.Exp, bias=0.0, scale=-0.5)
            znr = work.tile([1, LATENT], F32, tag="znr")
            nc.vector.tensor_scalar(znr, zc, zrs, None, op0=OP.mult)
            lnw = cpool.tile([1, LATENT], F32)
            nc.sync.dma_start(out=lnw, in_=lnw_d[:, :])
            lnb = cpool.tile([1, LATENT], F32)
            nc.sync.dma_start(out=lnb, in_=lnb_d[:, :])
            zsc = work.tile([1, LATENT], F32, tag="zsc")
            nc.vector.tensor_tensor(zsc, znr, lnw, op=OP.mult)
            zfin = work.tile([1, LATENT], F32, tag="zfin")
            nc.vector.tensor_tensor(zfin, zsc, lnb, op=OP.add)
            nc.sync.dma_start(out=out_d[:, :], in_=zfin)

    nc.compile()
    return nc


def prep_core_inputs(inputs, bi, si, n_layers=N_LAYERS, d_sh=D_INNER // G,
                     t_len=T, n_shards=G):
    """Host-side prep for one core = (batch bi, shard si)."""
    import ml_dtypes

    sl = slice(si * d_sh, (si + 1) * d_sh)
    l_in = (t_len - 1) * S_DS + K_DS

    x = np.asarray(inputs["x"], np.float32)
    xp = np.ascontiguousarray(x[bi, 0, :l_in].reshape(t_len, S_DS).T)

    conv_w = np.asarray(inputs["conv_w"], np.float32)
    dsw = np.ascontiguousarray(conv_w[:, 0, :].T)
    dsb = np.asarray(inputs["conv_b"], np.float32).reshape(D_MODEL, 1)

    norm_w = np.asarray(inputs["norm_w"], np.float32)[:n_layers]
    in_w = np.asarray(inputs["in_proj_w"], np.float32)[:n_layers]
    inw = np.empty((n_layers, D_MODEL, 2 * d_sh), np.float32)
    for l in range(n_layers):
        wl = in_w[l] * norm_w[l][None, :]
        rows = np.concatenate(
            [wl[sl, :], wl[D_INNER + si * d_sh: D_INNER + (si + 1) * d_sh, :]], 0)
        inw[l] = rows.T
    xpw_raw = np.asarray(inputs["x_proj_w"], np.float32)[:n_layers, :, sl].transpose(0, 2, 1)
    xpw = np.zeros((n_layers, d_sh, 96), np.float32)
    xpw[:, :, 0:DT_RANK] = xpw_raw[:, :, 0:DT_RANK]
    xpw[:, :, 64:96] = xpw_raw[:, :, DT_RANK:80]
    dtw = np.ascontiguousarray(
        np.asarray(inputs["dt_proj_w"], np.float32)[:n_layers, sl, :].transpose(0, 2, 1))
    outw = np.ascontiguousarray(
        np.asarray(inputs["out_proj_w"], np.float32)[:n_layers, :, sl].transpose(0, 2, 1))
    cvw = np.ascontiguousarray(np.asarray(inputs["conv1d_w"], np.float32)[:n_layers, sl, :])
    cvb = np.ascontiguousarray(np.asarray(inputs["conv1d_b"], np.float32)[:n_layers, sl, None])
    dtb = np.ascontiguousarray(np.asarray(inputs["dt_proj_b"], np.float32)[:n_layers, sl, None])
    A = np.ascontiguousarray(
        -np.exp(np.asarray(inputs["A_log"], np.float32))[:n_layers, sl, :])
    dsk = np.ascontiguousarray(np.asarray(inputs["D_skip"], np.float32)[:n_layers, sl, None])

    qid = (np.eye(P) / n_shards).astype(np.float32)
    idb = np.eye(P).astype(np.float32)

    norm_f = np.asarray(inputs["norm_f_w"], np.float32)
    proj_w = np.asarray(inputs["proj_w"], np.float32)
    pjw = np.ascontiguousarray(((proj_w * norm_f[None, :]) / t_len).T)
    pjb = np.asarray(inputs["proj_b"], np.float32).reshape(LATENT, 1)
    lnw = np.asarray(inputs["ln_w"], np.float32).reshape(1, LATENT)
    lnb = np.asarray(inputs["ln_b"], np.float32).reshape(1, LATENT)

    def bf(a):
        return np.ascontiguousarray(a.astype(ml_dtypes.bfloat16))

    return {
        "xds": xp, "dsw": dsw, "dsb": dsb,
        "inw": bf(inw), "xpw": bf(xpw), "dtw": bf(dtw), "outw": bf(outw),
        "cvw": cvw, "cvb": cvb, "dtb": dtb, "acol": A, "dsk": bf(dsk),
        "qid": qid, "idb": bf(idb),
        "ones": bf(np.ones((1, P), np.float32)),
        "onec": bf(np.ones((P, 1), np.float32)),
        "pjw": pjw, "pjb": pjb, "lnw": lnw, "lnb": lnb,
    }


def a_const_from_inputs(inputs, n_layers=N_LAYERS):
    A = -np.exp(np.asarray(inputs["A_log"], np.float64))[:n_layers]
    if np.allclose(A, A[:, :1, :], rtol=1e-6, atol=0):
        return [[float(A[l, 0, s]) for s in range(D_STATE)] for l in range(n_layers)]
    return None


_BUILT = {}


def kernel(**inputs) -> np.ndarray:
    from concourse.bass_utils import run_bass_kernel_spmd

    a_const = a_const_from_inputs(inputs)
    key = ("full", a_const is None)
    if key not in _BUILT:
        _BUILT[key] = build_bass(
            N_LAYERS, T, D_INNER // G,
            replica_groups=[[0, 1, 2, 3], [4, 5, 6, 7]],
            use_cc=True, a_const=a_const)
    nc = _BUILT[key]
    in_maps = [prep_core_inputs(inputs, c // G, c % G) for c in range(N_CORES)]
    res = run_bass_kernel_spmd(nc, in_maps, list(range(N_CORES)))
    out = np.zeros((B, LATENT), np.float32)
    out[0] = np.asarray(res.results[0]["head_out"]).reshape(LATENT)
    out[1] = np.asarray(res.results[G]["head_out"]).reshape(LATENT)
    return out



# revision 29
# speedup vs baseline: 99.9497x; 99.9497x over previous
import sys

sys.path.insert(0, "/opt/trn_rl_repo")

import numpy as np

# model dims (hardcoded per problem spec)
B = 2
L_IN = 135000
D_MODEL = 768
N_LAYERS = 24
D_INNER = 1536
D_STATE = 16
D_CONV = 4
DT_RANK = 48
K_DS = 32
S_DS = 32
LATENT = 64
T = (L_IN - K_DS) // S_DS + 1  # 4218

N_CORES = 8
G = 4                 # d_inner shards per batch group
TC = 512              # time chunk
SG = 2                # s-planes per scan group
P = 128


def _chunks(total, step):
    out = []
    t0 = 0
    while t0 < total:
        out.append((t0, min(step, total - t0)))
        t0 += step
    return out


def build_bass(n_layers, t_len, d_sh, replica_groups, use_cc, a_const=None):
    """Build the SPMD bass program (identical on every core; data differs)."""
    import concourse.bass as bass
    import concourse.bacc as bacc
    import concourse.tile as tile
    from concourse import mybir

    F32 = mybir.dt.float32
    BF16 = mybir.dt.bfloat16
    F16 = mybir.dt.float16
    AF = mybir.ActivationFunctionType
    OP = mybir.AluOpType

    DB = d_sh // P            # d-blocks per core
    NSG = D_STATE // SG       # scan groups
    SEG = SG * DB             # scan segments per group tile
    chunks = _chunks(t_len, TC)
    NCH = len(chunks)
    MB = D_MODEL // P         # 6

    nc = bacc.Bacc("TRN2", target_bir_lowering=False, debug=True)

    # ---------------- DRAM inputs ----------------
    xds_d = nc.dram_tensor("xds", [K_DS, t_len], F32, kind="ExternalInput")
    dsw_d = nc.dram_tensor("dsw", [K_DS, D_MODEL], F32, kind="ExternalInput")
    dsb_d = nc.dram_tensor("dsb", [D_MODEL, 1], F32, kind="ExternalInput")
    inw_d = nc.dram_tensor("inw", [n_layers, D_MODEL, 2 * d_sh], BF16, kind="ExternalInput")
    xpw_d = nc.dram_tensor("xpw", [n_layers, d_sh, 96], BF16, kind="ExternalInput")
    dtw_d = nc.dram_tensor("dtw", [n_layers, DT_RANK, d_sh], BF16, kind="ExternalInput")
    outw_d = nc.dram_tensor("outw", [n_layers, d_sh, D_MODEL], BF16, kind="ExternalInput")
    DBn = d_sh // P
    cdg_d = nc.dram_tensor("cdg", [n_layers, P, DBn * D_CONV * P], BF16,
                           kind="ExternalInput")
    cvb_d = nc.dram_tensor("cvb", [n_layers, d_sh, 1], F32, kind="ExternalInput")
    dtb_d = nc.dram_tensor("dtb", [n_layers, d_sh, 1], F32, kind="ExternalInput")
    acol_d = nc.dram_tensor("acol", [n_layers, d_sh, D_STATE], F32, kind="ExternalInput")
    dsk_d = nc.dram_tensor("dsk", [n_layers, d_sh, 1], BF16, kind="ExternalInput")
    qid_d = nc.dram_tensor("qid", [P, P], F32, kind="ExternalInput")   # I / n_shards
    idb_d = nc.dram_tensor("idb", [P, P], BF16, kind="ExternalInput")  # identity
    ones_d = nc.dram_tensor("ones", [1, P], BF16, kind="ExternalInput")
    onec_d = nc.dram_tensor("onec", [P, 1], BF16, kind="ExternalInput")
    pjw_d = nc.dram_tensor("pjw", [D_MODEL, LATENT], F32, kind="ExternalInput")
    pjb_d = nc.dram_tensor("pjb", [LATENT, 1], F32, kind="ExternalInput")
    lnw_d = nc.dram_tensor("lnw", [1, LATENT], F32, kind="ExternalInput")
    lnb_d = nc.dram_tensor("lnb", [1, LATENT], F32, kind="ExternalInput")

    out_d = nc.dram_tensor("head_out", [1, LATENT], F32, kind="ExternalOutput")

    with tile.TileContext(nc) as tc:
        import contextlib
        with contextlib.ExitStack() as ctx:
            wpool = ctx.enter_context(tc.tile_pool(name="wpool", bufs=2))
            cpool = ctx.enter_context(tc.tile_pool(name="cpool", bufs=1))
            work = ctx.enter_context(tc.tile_pool(name="work", bufs=2))
            lat = ctx.enter_context(tc.tile_pool(name="lat", bufs=2))
            spool = ctx.enter_context(tc.tile_pool(name="spool", bufs=1))
            mm = ctx.enter_context(tc.tile_pool(name="mm", bufs=2, space="PSUM"))
            sm = ctx.enter_context(tc.tile_pool(name="sm", bufs=1, space="PSUM"))
            ypsum = ctx.enter_context(tc.tile_pool(name="ypsum", bufs=1, space="PSUM"))
            dram = ctx.enter_context(tc.tile_pool(name="dram", bufs=1, space="DRAM"))

            # persistent constants
            idb = cpool.tile([P, P], BF16)
            nc.sync.dma_start(out=idb, in_=idb_d[:, :])
            qid = cpool.tile([P, P], F32)
            nc.sync.dma_start(out=qid, in_=qid_d[:, :])
            ones_r = cpool.tile([1, P], BF16)
            nc.sync.dma_start(out=ones_r, in_=ones_d[:, :])
            onec = cpool.tile([P, 1], BF16)
            nc.sync.dma_start(out=onec, in_=onec_d[:, :])
            epst = cpool.tile([P, 1], F32)
            nc.vector.memset(epst, 1e-5)
            zscol = cpool.tile([P, 1], F32)
            nc.vector.memset(zscol, 0.0)

            cc_in = [dram.tile([NCH, D_MODEL, TC], F32, name=f"cc_in{i}")
                     for i in range(2)]
            cc_out = [dram.tile([NCH, D_MODEL, TC], F32, name=f"cc_out{i}")
                      for i in range(2)]
            bc_dram = dram.tile([2 * D_STATE, t_len], BF16)
            tail_t0, tail_tw = chunks[-1]
            if tail_tw < TC:
                zt = work.tile([P, TC], F32, tag="dsout")
                nc.vector.memset(zt, 0.0)
                for bix in range(2):
                    for mb in range(MB):
                        nc.sync.dma_start(
                            out=cc_in[bix][NCH - 1, mb * P:(mb + 1) * P, tail_tw:TC],
                            in_=zt[:, :TC - tail_tw])
            zb_dram = dram.tile([LATENT, 1], F32)

            def rmsnorm_rs(h_c, tw, tag_suffix=""):
                """h_c [P, MB, tw] f32 -> rs broadcast psum tile [P, tw]."""
                sq = work.tile([P, MB, TC], BF16, tag="nrm")
                nc.scalar.activation(sq[:, :, :tw], h_c[:, :, :tw], AF.Square)
                msq = sm.tile([1, TC], F32, tag="msq")
                for kb in range(MB):
                    nc.tensor.matmul(msq[:, :tw], onec, sq[:, kb, :tw],
                                     start=(kb == 0), stop=(kb == MB - 1))
                lnm = work.tile([1, TC], F32, tag="lnm")
                nc.scalar.activation(lnm[:, :tw], msq[:, :tw], AF.Ln,
                                     bias=epst[:1, :], scale=1.0 / D_MODEL)
                rsr = work.tile([1, TC], BF16, tag="rsr")
                nc.scalar.activation(rsr[:, :tw], lnm[:, :tw], AF.Exp,
                                     bias=0.0, scale=-0.5)
                rsb = sm.tile([P, TC], F32, tag="rsb")
                nc.tensor.matmul(rsb[:, :tw], ones_r, rsr[:, :tw], start=True, stop=True)
                return rsb

            # ---------------- downsample conv -> cc_out[1] ----------------
            dsw = cpool.tile([K_DS, D_MODEL], F32)
            nc.sync.dma_start(out=dsw, in_=dsw_d[:, :])
            dsb = cpool.tile([P, MB], F32)
            nc.sync.dma_start(out=dsb, in_=dsb_d.rearrange("(mb p) one -> p (mb one)", p=P))
            for ci, (t0, tw) in enumerate(chunks):
                xc = work.tile([K_DS, TC], F32, tag="dsout")
                nc.sync.dma_start(out=xc[:, :tw], in_=xds_d[:, t0:t0 + tw])
                for mb in range(MB):
                    ps = mm.tile([P, TC], F32, tag="mm")
                    nc.tensor.matmul(ps[:, :tw], dsw[:, mb * P:(mb + 1) * P],
                                     xc[:, :tw], start=True, stop=True)
                    hsb = work.tile([P, TC], F32, tag="dsout")
                    nc.scalar.activation(hsb[:, :tw], ps[:, :tw], AF.Identity,
                                         bias=dsb[:, mb:mb + 1], scale=1.0)
                    nc.sync.dma_start(out=cc_out[1][ci, mb * P:(mb + 1) * P, :tw],
                                      in_=hsb[:, :tw])

            # ---------------- layers ----------------
            for l in range(n_layers):
                pp = l % 2
                prev = 1 - pp

                inw = wpool.tile([P, MB, 2 * d_sh], BF16, tag="inw")
                nc.sync.dma_start(out=inw, in_=inw_d[l].rearrange("(kb p) m -> p kb m", p=P))
                xpw = wpool.tile([P, DB, 96], BF16, tag="xpw")
                nc.sync.dma_start(out=xpw, in_=xpw_d[l].rearrange("(kb p) m -> p kb m", p=P))
                dtw = wpool.tile([DT_RANK, d_sh], BF16, tag="dtw")
                nc.sync.dma_start(out=dtw, in_=dtw_d[l, :, :])
                outw = wpool.tile([P, DB, D_MODEL], BF16, tag="outw")
                nc.sync.dma_start(out=outw, in_=outw_d[l].rearrange("(kb p) m -> p kb m", p=P))
                cdg = wpool.tile([P, DB, D_CONV, P], BF16, tag="cdg")
                nc.sync.dma_start(out=cdg, in_=cdg_d[l].rearrange("p (db j m) -> p db j m",
                                                                  db=DB, j=D_CONV))
                cvb = wpool.tile([P, DB], F32, tag="cvb")
                nc.sync.dma_start(out=cvb, in_=cvb_d[l].rearrange("(kb p) one -> p (kb one)", p=P))
                dtb = wpool.tile([P, DB], F32, tag="dtb")
                nc.sync.dma_start(out=dtb, in_=dtb_d[l].rearrange("(kb p) one -> p (kb one)", p=P))
                dsk = wpool.tile([P, DB], BF16, tag="dsk")
                nc.sync.dma_start(out=dsk, in_=dsk_d[l].rearrange("(kb p) one -> p (kb one)", p=P))
                if a_const is None:
                    acol = wpool.tile([P, DB, D_STATE], F32, tag="acol")
                    nc.sync.dma_start(out=acol,
                                      in_=acol_d[l].rearrange("(kb p) s -> p kb s", p=P))

                state = spool.tile([P, D_STATE * DB], F32, tag="state")
                nc.vector.memset(state, 0.0)

                prev_uraw = None
                for ci, (t0, tw) in enumerate(chunks):
                    # ---- A1: load h chunk (residual already folded in) ----
                    h_c = work.tile([P, MB, TC], F32, tag="h_c")
                    nc.sync.dma_start(out=h_c[:, :, :tw], in_=cc_out[prev][ci, :, :tw]
                                      .rearrange("(mb p) t -> p mb t", p=P))
                    # ---- A2: rmsnorm ----
                    rsb = rmsnorm_rs(h_c, tw)
                    normed = work.tile([P, MB, TC], BF16, tag="nrm")
                    rsb3 = bass.AP(tensor=rsb.tensor, offset=rsb.offset,
                                   ap=[rsb.ap[0], [0, MB], [1, tw]])
                    nc.vector.tensor_tensor(normed[:, :, :tw], h_c[:, :, :tw], rsb3,
                                            op=OP.mult)
                    # ---- A3: in_proj ----
                    uraw = work.tile([P, DB, D_CONV - 1 + TC], BF16, tag="uraw")
                    if ci == 0:
                        nc.vector.memset(uraw[:, :, 0:D_CONV - 1], 0.0)
                    else:
                        pw = chunks[ci - 1][1]
                        nc.vector.tensor_copy(uraw[:, :, 0:D_CONV - 1],
                                              prev_uraw[:, :, pw:pw + D_CONV - 1])
                    siluz = work.tile([P, DB, TC], BF16, tag="siluz")
                    for mb in range(2 * DB):
                        psm = mm.tile([P, TC], F32, tag="mm")
                        for kb in range(MB):
                            nc.tensor.matmul(psm[:, :tw],
                                             inw[:, kb, mb * P:(mb + 1) * P],
                                             normed[:, kb, :tw],
                                             start=(kb == 0), stop=(kb == MB - 1))
                        if mb < DB:
                            nc.scalar.copy(uraw[:, mb, D_CONV - 1:D_CONV - 1 + tw],
                                           psm[:, :tw])
                        else:
                            db = mb - DB
                            nc.scalar.activation(siluz[:, db, :tw], psm[:, :tw],
                                                 AF.Silu)
                    # ---- A4: causal depthwise conv (diag matmuls on PE) + silu ----
                    u_f = work.tile([P, DB, TC], BF16, tag="u_f")
                    for db in range(DB):
                        pc = mm.tile([P, TC], F32, tag="mm")
                        for j in range(D_CONV):
                            nc.tensor.matmul(pc[:, :tw], cdg[:, db, j, :],
                                             uraw[:, db, j:j + tw],
                                             start=(j == 0), stop=(j == D_CONV - 1))
                        nc.scalar.activation(u_f[:, db, :tw], pc[:, :tw], AF.Silu,
                                             bias=cvb[:, db:db + 1], scale=1.0)
                    # ---- A5: x_proj ----
                    psx = mm.tile([P, TC], F32, tag="mm")
                    for kb in range(DB):
                        nc.tensor.matmul(psx[0:96, :tw], xpw[:, kb, :], u_f[:, kb, :tw],
                                         start=(kb == 0), stop=(kb == DB - 1))
                    dtv = work.tile([64, TC], BF16, tag="dtv")
                    nc.scalar.copy(dtv[:, :tw], psx[0:64, :tw])
                    bcr = work.tile([2 * D_STATE, TC], BF16, tag="bcr")
                    nc.scalar.copy(bcr[:, :tw], psx[64:96, :tw])
                    nc.sync.dma_start(out=bc_dram[:, t0:t0 + tw], in_=bcr[:, :tw])
                    # ---- A6: dt_proj -> delta (softplus), w = delta*u ----
                    dlt = work.tile([P, DB, TC], BF16, tag="dlt")
                    esb = work.tile([P, DB, TC], BF16, tag="esb3")
                    for db in range(DB):
                        psd = mm.tile([P, TC], F32, tag="mm")
                        nc.tensor.matmul(psd[:, :tw], dtw[:, db * P:(db + 1) * P],
                                         dtv[0:DT_RANK, :tw], start=True, stop=True)
                        nc.scalar.activation(esb[:, db, :tw], psd[:, :tw], AF.Exp,
                                             bias=dtb[:, db:db + 1], scale=1.0)
                    nc.scalar.activation(dlt[:, :, :tw], esb[:, :, :tw], AF.Ln,
                                         bias=1.0, scale=1.0)
                    wmu = work.tile([P, DB, TC], BF16, tag="wmu")
                    nc.vector.tensor_tensor(wmu[:, :, :tw], dlt[:, :, :tw],
                                            u_f[:, :, :tw], op=OP.mult)
                    # ---- A7: scan ----
                    yp = ypsum.tile([P, DB * TC], F32, tag="yp")
                    for g in range(NSG):
                        s0 = g * SG
                        bbc = lat.tile([P, SG, TC], BF16, tag="bbc")
                        nc.sync.dma_start(
                            out=bbc[:, :, :tw],
                            in_=bass.AP(tensor=bc_dram.tensor,
                                        offset=bc_dram.offset + s0 * t_len + t0,
                                        ap=[[0, P], [t_len, SG], [1, tw]]))
                        cbc = lat.tile([P, SG, TC], BF16, tag="cbc")
                        nc.sync.dma_start(
                            out=cbc[:, :, :tw],
                            in_=bass.AP(tensor=bc_dram.tensor,
                                        offset=bc_dram.offset + (D_STATE + s0) * t_len + t0,
                                        ap=[[0, P], [t_len, SG], [1, tw]]))
                        av = lat.tile([P, SG, DB, 1 + TC], F32, tag="av")
                        nc.vector.memset(av[:, :, :, 0:1], 0.0)
                        for si in range(SG):
                            s = s0 + si
                            if a_const is not None:
                                nc.scalar.activation(av[:, si, :, 1:1 + tw],
                                                     dlt[:, :, :tw], AF.Exp,
                                                     bias=0.0, scale=float(a_const[l][s]))
                            else:
                                for db in range(DB):
                                    nc.scalar.activation(av[:, si, db, 1:1 + tw],
                                                         dlt[:, db, :tw], AF.Exp,
                                                         bias=0.0,
                                                         scale=acol[:, db, s:s + 1])
                        bv = lat.tile([P, SG, DB, 1 + TC], F16, tag="bv")
                        st_ap = bass.AP(tensor=state.tensor, offset=state.offset + s0 * DB,
                                        ap=[state.ap[0], [DB, SG], [1, DB], [0, 1]])
                        nc.vector.tensor_copy(bv[:, :, :, 0:1], st_ap)
                        wb = bass.AP(tensor=wmu.tensor, offset=wmu.offset,
                                     ap=[wmu.ap[0], [0, SG], [TC, DB], [1, tw]])
                        bb = bass.AP(tensor=bbc.tensor, offset=bbc.offset,
                                     ap=[bbc.ap[0], [TC, SG], [0, DB], [1, tw]])
                        nc.vector.tensor_tensor(bv[:, :, :, 1:1 + tw], wb, bb,
                                                op=OP.mult)
                        hv = lat.tile([P, SG, DB, 1 + TC], F16, tag="hv")
                        if tw == TC:
                            flat = SEG * (1 + TC)
                            sc_a = bass.AP(tensor=av.tensor, offset=av.offset,
                                           ap=[av.ap[0], [1, flat]])
                            sc_b = bass.AP(tensor=bv.tensor, offset=bv.offset,
                                           ap=[bv.ap[0], [1, flat]])
                            sc_h = bass.AP(tensor=hv.tensor, offset=hv.offset,
                                           ap=[hv.ap[0], [1, flat]])
                            nc.vector.tensor_tensor_scan(sc_h, sc_a, sc_b,
                                                         zscol[:, 0:1],
                                                         op0=OP.mult, op1=OP.add)
                        else:
                            for si in range(SG):
                                for db in range(DB):
                                    nc.vector.tensor_tensor_scan(
                                        hv[:, si, db, 0:1 + tw],
                                        av[:, si, db, 0:1 + tw],
                                        bv[:, si, db, 0:1 + tw], zscol[:, 0:1],
                                        op0=OP.mult, op1=OP.add)
                        st_ap2 = bass.AP(tensor=state.tensor, offset=state.offset + s0 * DB,
                                         ap=[state.ap[0], [DB, SG], [1, DB], [0, 1]])
                        nc.vector.tensor_copy(st_ap2, hv[:, :, :, tw:tw + 1])
                        yc = lat.tile([P, SG, DB, TC], BF16, tag="yc")
                        cb3 = bass.AP(tensor=cbc.tensor, offset=cbc.offset,
                                      ap=[cbc.ap[0], [TC, SG], [0, DB], [1, tw]])
                        nc.gpsimd.tensor_tensor(yc[:, :, :, :tw],
                                                hv[:, :, :, 1:1 + tw],
                                                cb3, op=OP.mult)
                        for si in range(SG):
                            s = s0 + si
                            for db in range(DB):
                                nc.tensor.matmul(
                                    yp[:, db * TC:db * TC + tw], idb,
                                    yc[:, si, db, :tw],
                                    start=(s == 0), stop=(s == D_STATE - 1),
                                    skip_group_check=True)
                    # ---- A8: gating ----
                    yg = work.tile([P, DB, TC], BF16, tag="dlt")
                    for db in range(DB):
                        g1 = work.tile([P, TC], F32, tag="esb")
                        nc.vector.scalar_tensor_tensor(
                            g1[:, :tw], u_f[:, db, :tw], dsk[:, db:db + 1],
                            yp[:, db * TC:db * TC + tw], op0=OP.mult, op1=OP.add)
                        nc.vector.tensor_tensor(yg[:, db, :tw], g1[:, :tw],
                                                siluz[:, db, :tw], op=OP.mult)
                    # ---- A9: out_proj + h/G via identity ----
                    for mb in range(MB):
                        pso = mm.tile([P, TC], F32, tag="mm")
                        for kb in range(DB):
                            nc.tensor.matmul(pso[:, :tw],
                                             outw[:, kb, mb * P:(mb + 1) * P],
                                             yg[:, kb, :tw],
                                             start=(kb == 0), stop=False,
                                             skip_group_check=True)
                        nc.tensor.matmul(pso[:, :tw], qid, h_c[:, mb, :tw],
                                         start=False, stop=True, skip_group_check=True)
                        oso = work.tile([P, TC], F32, tag="dsout")
                        nc.scalar.copy(oso[:, :tw], pso[:, :tw])
                        nc.sync.dma_start(out=cc_in[pp][ci, mb * P:(mb + 1) * P, :tw],
                                          in_=oso[:, :tw])
                    if use_cc:
                        nc.gpsimd.collective_compute(
                            "AllReduce", OP.add,
                            replica_groups=replica_groups,
                            ins=[cc_in[pp][ci].opt()],
                            outs=[cc_out[pp][ci].opt()],
                        )
                    else:
                        nc.sync.dma_start(out=cc_out[pp][ci, :, :],
                                          in_=cc_in[pp][ci, :, :])
                    prev_uraw = uraw

            # ---------------- head ----------------
            fin = (n_layers - 1) % 2
            pool_acc = spool.tile([P, MB], F32, tag="pool_acc")
            nc.vector.memset(pool_acc, 0.0)
            for ci, (t0, tw) in enumerate(chunks):
                h_c = work.tile([P, MB, TC], F32, tag="h_c")
                nc.sync.dma_start(out=h_c[:, :, :tw], in_=cc_out[fin][ci, :, :tw]
                                  .rearrange("(mb p) t -> p mb t", p=P))
                rsb = rmsnorm_rs(h_c, tw)
                tmp = work.tile([P, MB, TC], BF16, tag="nrm")
                rsb3 = bass.AP(tensor=rsb.tensor, offset=rsb.offset,
                               ap=[rsb.ap[0], [0, MB], [1, tw]])
                nc.vector.tensor_tensor(tmp[:, :, :tw], h_c[:, :, :tw], rsb3, op=OP.mult)
                pr = work.tile([P, MB], F32, tag="pr")
                nc.vector.tensor_reduce(pr, tmp[:, :, :tw], axis=mybir.AxisListType.X,
                                        op=OP.add)
                nc.vector.tensor_tensor(pool_acc, pool_acc, pr, op=OP.add)
            pjw = cpool.tile([P, MB, LATENT], F32)
            nc.sync.dma_start(out=pjw, in_=pjw_d.rearrange("(mb p) m -> p mb m", p=P))
            pjb = cpool.tile([LATENT, 1], F32)
            nc.sync.dma_start(out=pjb, in_=pjb_d[:, :])
            psz = sm.tile([LATENT, 1], F32, tag="msq")
            for kb in range(MB):
                nc.tensor.matmul(psz, pjw[:, kb, :], pool_acc[:, kb:kb + 1],
                                 start=(kb == 0), stop=(kb == MB - 1))
            zcol = work.tile([LATENT, 1], F32, tag="zcol")
            nc.scalar.activation(zcol, psz, AF.Identity, bias=pjb[:, :], scale=1.0)
            nc.sync.dma_start(out=zb_dram[:, :], in_=zcol)
            zrow = work.tile([1, LATENT], F32, tag="zrow")
            nc.sync.dma_start(out=zrow,
                              in_=bass.AP(tensor=zb_dram.tensor, offset=zb_dram.offset,
                                          ap=[[0, 1], [1, LATENT]]))
            zmu = work.tile([1, 1], F32, tag="zmu")
            nc.vector.tensor_reduce(zmu, zrow, axis=mybir.AxisListType.X, op=OP.add)
            zmm = work.tile([1, 1], F32, tag="zmm")
            nc.scalar.activation(zmm, zmu, AF.Identity, bias=0.0, scale=1.0 / LATENT)
            zc = work.tile([1, LATENT], F32, tag="zc")
            nc.vector.tensor_scalar(zc, zrow, zmm, None, op0=OP.subtract)
            zsq = work.tile([1, LATENT], F32, tag="zsq")
            nc.scalar.activation(zsq, zc, AF.Square)
            zvar = work.tile([1, 1], F32, tag="zvar")
            nc.vector.tensor_reduce(zvar, zsq, axis=mybir.AxisListType.X, op=OP.add)
            zln = work.tile([1, 1], F32, tag="zln")
            nc.scalar.activation(zln, zvar, AF.Ln, bias=epst[:1, :], scale=1.0 / LATENT)
            zrs = work.tile([1, 1], F32, tag="zrs")
            nc.scalar.activation(zrs, zln, AF.Exp, bias=0.0, scale=-0.5)
            znr = work.tile([1, LATENT], F32, tag="znr")
            nc.vector.tensor_scalar(znr, zc, zrs, None, op0=OP.mult)
            lnw = cpool.tile([1, LATENT], F32)
            nc.sync.dma_start(out=lnw, in_=lnw_d[:, :])
            lnb = cpool.tile([1, LATENT], F32)
            nc.sync.dma_start(out=lnb, in_=lnb_d[:, :])
            zsc = work.tile([1, LATENT], F32, tag="zsc")
            nc.vector.tensor_tensor(zsc, znr, lnw, op=OP.mult)
            zfin = work.tile([1, LATENT], F32, tag="zfin")
            nc.vector.tensor_tensor(zfin, zsc, lnb, op=OP.add)
            nc.sync.dma_start(out=out_d[:, :], in_=zfin)

    nc.compile()
    return nc


def prep_core_inputs(inputs, bi, si, n_layers=N_LAYERS, d_sh=D_INNER // G,
                     t_len=T, n_shards=G):
    """Host-side prep for one core = (batch bi, shard si)."""
    import ml_dtypes

    sl = slice(si * d_sh, (si + 1) * d_sh)
    l_in = (t_len - 1) * S_DS + K_DS

    x = np.asarray(inputs["x"], np.float32)
    xp = np.ascontiguousarray(x[bi, 0, :l_in].reshape(t_len, S_DS).T)

    conv_w = np.asarray(inputs["conv_w"], np.float32)
    dsw = np.ascontiguousarray(conv_w[:, 0, :].T)
    dsb = np.asarray(inputs["conv_b"], np.float32).reshape(D_MODEL, 1)

    norm_w = np.asarray(inputs["norm_w"], np.float32)[:n_layers]
    in_w = np.asarray(inputs["in_proj_w"], np.float32)[:n_layers]
    inw = np.empty((n_layers, D_MODEL, 2 * d_sh), np.float32)
    for l in range(n_layers):
        wl = in_w[l] * norm_w[l][None, :]
        rows = np.concatenate(
            [wl[sl, :], wl[D_INNER + si * d_sh: D_INNER + (si + 1) * d_sh, :]], 0)
        inw[l] = rows.T
    xpw_raw = np.asarray(inputs["x_proj_w"], np.float32)[:n_layers, :, sl].transpose(0, 2, 1)
    xpw = np.zeros((n_layers, d_sh, 96), np.float32)
    xpw[:, :, 0:DT_RANK] = xpw_raw[:, :, 0:DT_RANK]
    xpw[:, :, 64:96] = xpw_raw[:, :, DT_RANK:80]
    dtw = np.ascontiguousarray(
        np.asarray(inputs["dt_proj_w"], np.float32)[:n_layers, sl, :].transpose(0, 2, 1))
    outw = np.ascontiguousarray(
        np.asarray(inputs["out_proj_w"], np.float32)[:n_layers, :, sl].transpose(0, 2, 1))
    cvw = np.ascontiguousarray(np.asarray(inputs["conv1d_w"], np.float32)[:n_layers, sl, :])
    DBn = d_sh // P
    cdg = np.zeros((n_layers, P, DBn, D_CONV, P), np.float32)
    kk = np.arange(P)
    for l in range(n_layers):
        for db in range(DBn):
            for j in range(D_CONV):
                cdg[l, kk, db, j, kk] = cvw[l, db * P + kk, j]
    cdg = cdg.reshape(n_layers, P, DBn * D_CONV * P)
    cvb = np.ascontiguousarray(np.asarray(inputs["conv1d_b"], np.float32)[:n_layers, sl, None])
    dtb = np.ascontiguousarray(np.asarray(inputs["dt_proj_b"], np.float32)[:n_layers, sl, None])
    A = np.ascontiguousarray(
        -np.exp(np.asarray(inputs["A_log"], np.float32))[:n_layers, sl, :])
    dsk = np.ascontiguousarray(np.asarray(inputs["D_skip"], np.float32)[:n_layers, sl, None])

    qid = (np.eye(P) / n_shards).astype(np.float32)
    idb = np.eye(P).astype(np.float32)

    norm_f = np.asarray(inputs["norm_f_w"], np.float32)
    proj_w = np.asarray(inputs["proj_w"], np.float32)
    pjw = np.ascontiguousarray(((proj_w * norm_f[None, :]) / t_len).T)
    pjb = np.asarray(inputs["proj_b"], np.float32).reshape(LATENT, 1)
    lnw = np.asarray(inputs["ln_w"], np.float32).reshape(1, LATENT)
    lnb = np.asarray(inputs["ln_b"], np.float32).reshape(1, LATENT)

    def bf(a):
        return np.ascontiguousarray(a.astype(ml_dtypes.bfloat16))

    return {
        "xds": xp, "dsw": dsw, "dsb": dsb,
        "inw": bf(inw), "xpw": bf(xpw), "dtw": bf(dtw), "outw": bf(outw),
        "cdg": bf(cdg), "cvb": cvb, "dtb": dtb, "acol": A, "dsk": bf(dsk),
        "qid": qid, "idb": bf(idb),
        "ones": bf(np.ones((1, P), np.float32)),
        "onec": bf(np.ones((P, 1), np.float32)),
        "pjw": pjw, "pjb": pjb, "lnw": lnw, "lnb": lnb,
    }


def a_const_from_inputs(inputs, n_layers=N_LAYERS):
    A = -np.exp(np.asarray(inputs["A_log"], np.float64))[:n_layers]
    if np.allclose(A, A[:, :1, :], rtol=1e-6, atol=0):
        return [[float(A[l, 0, s]) for s in range(D_STATE)] for l in range(n_layers)]
    return None


_BUILT = {}


def kernel(**inputs) -> np.ndarray:
    from concourse.bass_utils import run_bass_kernel_spmd

    a_const = a_const_from_inputs(inputs)
    key = ("full", a_const is None)
    if key not in _BUILT:
        _BUILT[key] = build_bass(
            N_LAYERS, T, D_INNER // G,
            replica_groups=[[0, 1, 2, 3], [4, 5, 6, 7]],
            use_cc=True, a_const=a_const)
    nc = _BUILT[key]
    in_maps = [prep_core_inputs(inputs, c // G, c % G) for c in range(N_CORES)]
    res = run_bass_kernel_spmd(nc, in_maps, list(range(N_CORES)))
    out = np.zeros((B, LATENT), np.float32)
    out[0] = np.asarray(res.results[0]["head_out"]).reshape(LATENT)
    out[1] = np.asarray(res.results[G]["head_out"]).reshape(LATENT)
    return out

